# revision 26
# baseline (speedup 1.0000x reference)
"""IsoGMM loss kernel for 8 Trainium2 NeuronCores.

loss = mean_{n,k} r[n,k] * ||X[n] - mus[k]||^2

Decomposition (the loss folds into two accumulated PE matmuls per core):
  sum_{n,k} r*d2 = T1 + T2 - 2*T3
    T1 = sum_n xsq_n * R_n        (xsq_n = ||X[n]||^2, R_n = sum_k r[n,k])
    T2 = sum_k musq_k * C_k       (C_k = sum_n r[n,k])
    T3 = sum_{k,d} mus[k,d] * M[k,d],  M = r.T @ X

Host augments X rows to width 129: [X | 1]. Per pair of 128-row
segments, two DoubleRow matmuls (256-row contraction each):
  ps [64,129] += sum_i r_pair[:,i,:].T @ [X | 1]_pair[:,i,:]
  ps2[64,129] += sum_i r_pair[:,i,:].T @ ([X | 1]^2)_pair[:,i,:]
ps cols 0:128 = M, col 128 = C_k; sum_d ps2[k,d] = sum_n r[n,k]*
(xsq_n + 1) - the T1 ingredient, with the PE doing the per-row reduce
for free as part of the contraction. The stray +1 per row (squared
ones-column) is cancelled exactly by the musq-1 coefficient in the
host-side combine.

The elementwise square is the only remaining vector-engine pass; every
X element must cross a multiplier once and the PE can't square, so the
pass is split across DVE (~1.4ns/elem fp8), GPSIMD (~2.1ns/elem) and -
experimentally - the scalar/ACT engine per the SQ_* chunk maps.

X and r are shipped in fp8 e4m3 (the 2e-2 rel-err budget dwarfs fp8
noise, simulated+measured ~4e-3): quarters HBM traffic vs f32 and
enables the PE DoubleRow perf mode. X rides the sync-issued DMA queue,
r the scalar-issued one.

Sharding: data-parallel over N, 16384 rows per core. Each SBUF
partition holds 128 *contiguous* rows (row order is irrelevant for
every term), so every DMA is perfectly contiguous per partition. Chunk
sizes ramp up (fast pipeline fill after the ~7us engine-init preamble)
and back down (short tail chain: the last chunk's square+matmuls gate
the out DMA).
"""

import numpy as np
import ml_dtypes

import concourse.bass as bass
import concourse.mybir as mybir
import concourse.tile as tile
from concourse import bacc
from concourse.bass_utils import run_bass_kernel_spmd

N, K, D = 131072, 64, 128
NCORES = 8
W = D + 1            # augmented row width: 128 data + ones
NS = N // NCORES     # rows per core
RPP = NS // 128      # rows per SBUF partition (= segments per core)
# The DMA queues are packet-rate-bound (~20ns per 128-partition line
# regardless of size, saturating ~230 GB/s only at 4KB lines), so
# chunks are as big as possible: 32 segs = 4128B lines. Total X
# packets = 128 * n_chunks, split over two queues.
CHUNK_SIZES = (16, 32, 32, 32, 16)
# Every chunk's square splits three ways (DVE/ACT/GPSIMD slices, even-
# aligned) so the engines track DMA arrivals with no backlog and the
# trailing matmuls can start on a finished slice early. Shares follow
# measured rates net of each engine's descriptor-issue load:
# DVE ~41%, ACT ~36%, GPSIMD the rest.
SQ_DVE_FRAC, SQ_ACT_FRAC = 0.41, 0.36
# r ships on the sync engine's (slower) queue - it only has to beat
# the matmul stream, not the squares - with a small first chunk so the
# first pairs aren't starved.
R_CHUNKS = ((0, 16), (16, 48), (64, 64))  # (start_seg, n_segs)

FP8 = mybir.dt.float8e4
NP_FP8 = ml_dtypes.float8_e4m3


def build_nc(rpp=RPP, chunk_sizes=CHUNK_SIZES):
    segs = rpp
    assert sum(chunk_sizes) == segs
    xf = rpp * W
    rf = rpp * K
    f32 = mybir.dt.float32
    DR = mybir.MatmulPerfMode.DoubleRow

    # Bacc (not plain Bass): its compile() splits sync waits to satisfy
    # TRN2's 1-wait-per-instruction limit, which walrus enforces.
    nc = bacc.Bacc("TRN2", target_bir_lowering=False, debug=False)
    xp = nc.dram_tensor("xp", [128, xf], FP8, kind="ExternalInput")
    rp = nc.dram_tensor("rp", [128, rf], FP8, kind="ExternalInput")
    out = nc.dram_tensor("out", [K, 2 * W], f32, kind="ExternalOutput")

    with (
        tile.TileContext(nc) as tc,
        # Every chunk gets its own resident buffer (~41KB/partition total,
        # well under the 208KB budget): the DMA queue never stalls waiting
        # for compute to release a tile.
        tc.tile_pool(name="xb", bufs=len(CHUNK_SIZES)) as xpool,
        tc.tile_pool(name="rb", bufs=len(R_CHUNKS)) as rpool,
        tc.tile_pool(name="scr", bufs=len(CHUNK_SIZES)) as spool,
        tc.tile_pool(name="one", bufs=1) as onepool,
        tc.tile_pool(name="ps", bufs=2, space="PSUM") as pspool,
    ):
        ps = pspool.tile([K, W], f32, tag="ps")
        ps2 = pspool.tile([K, W], f32, tag="ps2")

        r_tiles = []
        for rs, rn in R_CHUNKS:
            rt = rpool.tile([128, rn * K], FP8, tag=f"r{rs}")
            nc.sync.dma_start(out=rt, in_=rp[:, rs * K:(rs + rn) * K])
            r_tiles.append((rs, rn, rt.rearrange("p (s k) -> p s k", k=K)))

        def r_pair(s):
            for rs, rn, r3 in r_tiles:
                if rs <= s < rs + rn:
                    return r3[:, s - rs:s - rs + 2, :]
            raise AssertionError(s)

        s0 = 0
        for c, spc in enumerate(chunk_sizes):
            xt = xpool.tile([128, spc * W], FP8, tag="x")
            # X rides the two fast DMA queues (gpsimd/scalar; the sync
            # engine's queue measured ~2x slower - its semaphore
            # choreography competes with queue service - so it carries
            # only r and the output). Descriptors all issue before the
            # owning engine's compute, which is arrival-gated anyway.
            xeng = nc.gpsimd if c % 2 == 0 else nc.scalar
            xeng.dma_start(out=xt, in_=xp[:, s0 * W:(s0 + spc) * W])

            x3 = xt.rearrange("p (s w) -> p s w", w=W)

            sq = spool.tile([128, spc * W], FP8, tag="sq")
            sq3 = sq.rearrange("p (s w) -> p s w", w=W)
            nd = 2 * round(SQ_DVE_FRAC * spc / 2)
            na = 2 * round(SQ_ACT_FRAC * spc / 2)
            for a0, a1, eng in (
                (0, nd, nc.vector),
                (nd, nd + na, nc.scalar),
                (nd + na, spc, nc.gpsimd),
            ):
                if a1 <= a0:
                    continue
                if eng is nc.scalar:
                    eng.square(sq[:, a0 * W:a1 * W], xt[:, a0 * W:a1 * W])
                else:
                    eng.tensor_mul(
                        sq[:, a0 * W:a1 * W],
                        xt[:, a0 * W:a1 * W],
                        xt[:, a0 * W:a1 * W],
                    )

            for j in range(0, spc, 2):
                s = s0 + j
                lhsT = r_pair(s)
                nc.tensor.matmul(
                    ps,
                    lhsT=lhsT,
                    rhs=x3[:, j:j + 2, :],
                    start=(s == 0),
                    stop=(s == segs - 2),
                    perf_mode=DR,
                )
                nc.tensor.matmul(
                    ps2,
                    lhsT=lhsT,
                    rhs=sq3[:, j:j + 2, :],
                    start=(s == 0),
                    stop=(s == segs - 2),
                    perf_mode=DR,
                )
            s0 += spc

        # Ship both accumulated [K, W] panels; the final weighted sum is
        # part of host-side unsharding.
        osb = onepool.tile([K, 2 * W], f32)
        nc.vector.tensor_copy(osb[:, 0:W], ps)
        nc.vector.tensor_copy(osb[:, W:2 * W], ps2)
        nc.sync.dma_start(out=out[:, :], in_=osb)

    nc.compile()
    return nc


def make_in_maps(X, r, mus, ncores=NCORES):
    X = np.ascontiguousarray(np.asarray(X, dtype=np.float32))
    r = np.ascontiguousarray(np.asarray(r, dtype=np.float32))
    n = X.shape[0]
    ns = n // ncores

    Xb = X.astype(NP_FP8)
    rb = r.astype(NP_FP8)

    in_maps = []
    for i in range(ncores):
        Xa = np.empty((ns, W), NP_FP8)
        Xa[:, :D] = Xb[i * ns:(i + 1) * ns]
        Xa[:, D] = 1.0
        in_maps.append(
            {
                "xp": np.ascontiguousarray(Xa.reshape(128, (ns // 128) * W)),
                "rp": np.ascontiguousarray(
                    rb[i * ns:(i + 1) * ns].reshape(128, (ns // 128) * K)
                ),
            }
        )
    return in_maps


def combine_outputs(results, mus):
    """Unshard: weighted sum of each core's panels -> mean."""
    mus = np.asarray(mus, dtype=np.float32)
    musq = (mus.astype(np.float64) ** 2).sum(1)
    # col 128 coefficient is musq-1: every row's T1 contribution (the
    # ps2 row-sum) carries a stray +1 from the squared ones-column,
    # cancelled exactly by the -1 on C_k here.
    ma = np.concatenate(
        [-2.0 * mus.astype(np.float64), musq[:, None] - 1.0], axis=1
    )
    total = 0.0
    for res in results:
        panel = res["out"].astype(np.float64)
        total += float((ma * panel[:, :W]).sum()) + float(panel[:, W:].sum())
    return np.array(total / (N * K), dtype=np.float32)


def kernel(X, r, mus):
    nc = build_nc()
    in_maps = make_in_maps(X, r, mus)
    res = run_bass_kernel_spmd(nc, in_maps, list(range(NCORES)))
    return combine_outputs(res.results[:NCORES], mus)
